# revision 4
# baseline (speedup 1.0000x reference)
"""APPNP (GCN-normalized personalized-pagerank propagation) on 8 Trainium2 NeuronCores.

Design:
- MLP h = relu(x@W1+b1)@W2+b2 on-device (PE), tokens sharded 8 ways.
- Propagation in "u-space" (u = dinv*z):
      u_next = (1-a)*dinv^2 * (gather_sum(u) + u_self) + a*dinv*h
  so each step needs only an unweighted gather+segment-sum of u rows.
- Nodes permuted by descending in-degree, striped across cores -> identical
  ELL schedule on all cores (single SPMD program).
- u table in HBM as [25216, 256] f32 super-rows (4 nodes / 1KB each);
  dma_gather descriptors are latency-bound so 1KB costs the same as 256B and
  int16 indices cover all nodes via super-row ids.  Per-slot 4-wide bf16
  masks (per-core data) select the right 256B section in masked DVE reduces.
- Ping-pong tables + AllGather of per-core shards each step.
"""

import numpy as np

N_NODES = 100000
IN_CH, HID_CH, OUT_CH = 512, 256, 48
K_STEPS = 10
ALPHA = 0.1

C = 8                 # cores
S_PER_CORE = 12544    # 98 * 128 slots per core
NB = 98               # buckets (128 dst lanes each) per core
NLANE = 128
R_TOT = C * S_PER_CORE          # 100352 table node rows
NSUP = 25216                    # super rows (4 node rows each) incl. zero pad
ZSUP = 25100                    # an all-zero super row used for ELL padding
F64 = 64                        # table row width in f32 (48 data + 16 zero)
CHUNK_SLOTS = 32                # gather ring chunk (slots of 1KB)
MAX_D = 32                      # max ELL slots per bucket piece


def _build_schedule(indeg_sorted):
    pieces = []  # (bucket j, D_piece)
    for j in range(NB):
        D = int(indeg_sorted[1024 * j]) - 1  # in-edges only (indeg includes +1)
        left = max(D, 0)
        while left > 0:
            d = min(left, MAX_D)
            pieces.append((j, d))
            left -= d
    chunks = []
    cur, cur_slots = [], 0
    for pi, (j, d) in enumerate(pieces):
        if cur_slots + d > CHUNK_SLOTS:
            chunks.append(cur)
            cur, cur_slots = [], 0
        cur.append(pi)
        cur_slots += d
    if cur:
        chunks.append(cur)
    return pieces, chunks


def _preprocess(edge_index):
    import ml_dtypes
    src = np.asarray(edge_index[0], dtype=np.int64)
    dst = np.asarray(edge_index[1], dtype=np.int64)
    indeg = np.bincount(dst, minlength=N_NODES).astype(np.int64) + 1

    order = np.argsort(-indeg, kind="stable")        # rank -> old node id
    rank_of = np.empty(N_NODES, dtype=np.int64)
    rank_of[order] = np.arange(N_NODES)

    indeg_sorted = indeg[order]
    pieces, chunks = _build_schedule(indeg_sorted)

    src_rank = rank_of[src]
    dst_rank = rank_of[dst]
    src_row = (src_rank % C) * S_PER_CORE + (src_rank // C)
    e_core = dst_rank % C
    e_slot = dst_rank // C

    ekey = e_core * S_PER_CORE + e_slot
    eorder = np.argsort(ekey, kind="stable")
    ekey_s = ekey[eorder]
    srow_s = src_row[eorder]
    counts = np.bincount(ekey_s, minlength=C * S_PER_CORE)
    offs = np.zeros(C * S_PER_CORE + 1, dtype=np.int64)
    np.cumsum(counts, out=offs[1:])

    tot_cols = sum(d for (_, d) in pieces)

    ell_sup = np.full((C, NLANE, tot_cols), ZSUP, dtype=np.int16)
    ell_cls = np.zeros((C, NLANE, tot_cols), dtype=np.int8)
    ell_valid = np.zeros((C, NLANE, tot_cols), dtype=bool)

    # vectorized ELL fill: for each (core, bucket-piece) place up to d edges
    piece_col = []
    col0 = 0
    consumed = np.zeros((C, NB * NLANE), dtype=np.int64)
    for (j, d) in pieces:
        piece_col.append(col0)
        slots = j * NLANE + np.arange(NLANE)
        for c in range(C):
            keys = c * S_PER_CORE + slots
            used = consumed[c, slots]
            st = offs[keys] + used
            cnt = np.clip(counts[keys] - used, 0, d)
            # flatten the (lane, k<cnt) pairs
            maxc = int(cnt.max()) if cnt.size else 0
            if maxc > 0:
                k = np.arange(maxc)
                lane_i, k_i = np.nonzero(k[None, :] < cnt[:, None])
                rows = srow_s[st[lane_i] + k_i]
                ell_sup[c, lane_i, col0 + k_i] = (rows >> 2).astype(np.int16)
                ell_cls[c, lane_i, col0 + k_i] = (rows & 3).astype(np.int8)
                ell_valid[c, lane_i, col0 + k_i] = True
            consumed[c, slots] = used + cnt
        col0 += d
    assert col0 == tot_cols

    # per-(core, lane, bucket) in-degree for device-side dinv
    deg_cls = np.zeros((C, NLANE, NB), dtype=np.float32)
    for c in range(C):
        s = np.arange(S_PER_CORE)
        r = 8 * s + c
        d = np.where(r < N_NODES, indeg[order[np.minimum(r, N_NODES - 1)]], 1)
        deg_cls[c] = d.reshape(NB, NLANE).T.astype(np.float32)

    onehot = (ell_cls[..., None] == np.arange(4, dtype=np.int8)[None, None, None, :])
    onehot = onehot & ell_valid[..., None]
    masks = onehot.astype(np.float32).astype(ml_dtypes.bfloat16)  # [C,128,tot,4]

    # wrapped int16 index stream; chunk layout
    chunk_meta = []
    idx_cols_total = 0
    for ch in chunks:
        slots = sum(pieces[pi][1] for pi in ch)
        chunk_meta.append((slots, ch, idx_cols_total))
        idx_cols_total += slots * 8
    idx_wrapped = np.zeros((C, NLANE, idx_cols_total), dtype=np.int16)
    pp16 = np.arange(NLANE) % 16
    for c in range(C):
        for (slots, ch, colbase) in chunk_meta:
            flat = np.empty(slots * NLANE, dtype=np.int16)
            m0 = 0
            for pi in ch:
                j, d = pieces[pi]
                pc = piece_col[pi]
                seg = ell_sup[c, :, pc:pc + d]           # [128, d]
                flat[m0 * NLANE:(m0 + d) * NLANE] = seg.T.reshape(-1)
                m0 += d
            cols = slots * 8
            col_idx = np.arange(cols)
            w = flat[col_idx[None, :] * 16 + pp16[:, None]]   # [128, cols]
            idx_wrapped[c, :, colbase:colbase + cols] = w
    return dict(order=order, indeg=indeg, pieces=pieces, piece_col=piece_col,
                chunk_meta=chunk_meta, tot_cols=tot_cols,
                masks=masks, idx_wrapped=idx_wrapped, deg=deg_cls)


def _build_program(pre):
    import concourse.bacc as bacc
    import concourse.tile as tile
    import concourse.mybir as mybir
    from concourse import library_config

    pieces = pre["pieces"]
    piece_col = pre["piece_col"]
    chunk_meta = pre["chunk_meta"]
    tot_cols = pre["tot_cols"]
    dt = mybir.dt
    AF = mybir.ActivationFunctionType
    OP = mybir.AluOpType

    nc = bacc.Bacc("TRN2", target_bir_lowering=False, debug=False, num_devices=C)

    xt_in = nc.dram_tensor("xt", [NB, 4, 128, 128], dt.float32, kind="ExternalInput")
    w1_in = nc.dram_tensor("w1", [IN_CH, HID_CH], dt.float32, kind="ExternalInput")
    b1_in = nc.dram_tensor("b1", [128, HID_CH], dt.float32, kind="ExternalInput")
    w2_in = nc.dram_tensor("w2", [HID_CH, OUT_CH], dt.float32, kind="ExternalInput")
    b2_in = nc.dram_tensor("b2", [128, OUT_CH], dt.float32, kind="ExternalInput")
    deg_in = nc.dram_tensor("deg", [NLANE, NB], dt.float32, kind="ExternalInput")
    msk_in = nc.dram_tensor("msk", [NLANE, tot_cols * 4], dt.bfloat16, kind="ExternalInput")
    idx_in = nc.dram_tensor("idx", [NLANE, pre["idx_wrapped"].shape[2]], dt.int16, kind="ExternalInput")
    id_in = nc.dram_tensor("ident", [128, 128], dt.float32, kind="ExternalInput")
    out_d = nc.dram_tensor("out", [S_PER_CORE, OUT_CH], dt.float32, kind="ExternalOutput")

    tabs = [nc.dram_tensor(f"tab{i}", [NSUP, 256], dt.float32, kind="Internal",
                           addr_space="Shared") for i in range(2)]
    bounce = nc.dram_tensor("bounce", [S_PER_CORE, F64], dt.float32, kind="Internal")

    with tile.TileContext(nc) as tc:
        with tc.tile_pool(name="main", bufs=1) as pool, \
             tc.tile_pool(name="ring", bufs=2) as ring, \
             tc.tile_pool(name="psum", bufs=2, space="PSUM") as psp:
            nc.gpsimd.load_library(library_config.mlp)

            u_t = pool.tile([NLANE, NB, OUT_CH], dt.float32)
            r_t = pool.tile([NLANE, NB, OUT_CH], dt.float32)
            ahd_t = pool.tile([NLANE, NB, OUT_CH], dt.float32)
            s2f_t = pool.tile([NLANE, NB, OUT_CH], dt.float32)
            msk_t = pool.tile([NLANE, tot_cols, 4], dt.bfloat16)
            deg_t = pool.tile([NLANE, NB], dt.float32)
            dinv_t = pool.tile([NLANE, NB], dt.float32)
            dinv2_t = pool.tile([NLANE, NB], dt.float32)
            sdeg_t = pool.tile([NLANE, NB], dt.float32)
            w1_t = pool.tile([128, 4, HID_CH], dt.float32)
            w2_t = pool.tile([128, 2, OUT_CH], dt.float32)
            b1_t = pool.tile([128, HID_CH], dt.float32)
            b2_t = pool.tile([128, OUT_CH], dt.float32)
            ident_t = pool.tile([128, 128], dt.float32)
            ztile = pool.tile([NLANE, 1568], dt.float32)

            nc.sync.dma_start(msk_t[:].rearrange("p s q -> p (s q)"), msk_in.ap())
            nc.sync.dma_start(deg_t[:], deg_in.ap())
            nc.sync.dma_start(w1_t[:], w1_in.ap().rearrange("(c p) h -> p c h", p=128))
            nc.sync.dma_start(w2_t[:], w2_in.ap().rearrange("(c p) h -> p c h", p=128))
            nc.sync.dma_start(b1_t[:], b1_in.ap())
            nc.sync.dma_start(b2_t[:], b2_in.ap())
            nc.sync.dma_start(ident_t[:], id_in.ap())

            # zero the 128 pad super-rows of both tables; zero bounce pad cols
            nc.vector.memset(ztile[:], 0.0)
            for tab in tabs:
                nc.sync.dma_start(tab.ap()[NSUP - 128:, :], ztile[:, :256])
            nc.sync.dma_start(
                bounce.ap().rearrange("(g p) f -> p g f", p=128)[:, :, OUT_CH:],
                ztile[:, :NB * (F64 - OUT_CH)].rearrange("p (g f) -> p g f", f=F64 - OUT_CH))

            nc.scalar.activation(sdeg_t[:], deg_t[:], AF.Sqrt)
            nc.vector.reciprocal(dinv2_t[:], deg_t[:])
            nc.vector.reciprocal(dinv_t[:], sdeg_t[:])

            # ---- MLP -> u0 = dinv * h ----
            for g in range(NB):
                xt_g = ring.tile([128, 4, 128], dt.float32, tag="xt")
                nc.sync.dma_start(xt_g[:], xt_in.ap()[g].rearrange("c p t -> p c t"))
                ps1 = psp.tile([128, HID_CH], dt.float32, tag="ps1")
                for cch in range(4):
                    nc.tensor.matmul(ps1[:], lhsT=xt_g[:, cch, :], rhs=w1_t[:, cch, :],
                                     start=(cch == 0), stop=(cch == 3))
                h1 = ring.tile([128, HID_CH], dt.float32, tag="h1")
                nc.vector.tensor_tensor(out=h1[:], in0=ps1[:],
                                        in1=b1_t[:],
                                        op=OP.add)
                nc.vector.tensor_scalar_max(h1[:], h1[:], 0.0)
                ps2 = psp.tile([128, OUT_CH], dt.float32, tag="ps2")
                for cch in range(2):
                    pT = psp.tile([128, 128], dt.float32, tag="pT")
                    nc.tensor.transpose(out=pT[:], in_=h1[:, cch * 128:(cch + 1) * 128],
                                        identity=ident_t[:])
                    h1T = ring.tile([128, 128], dt.float32, tag="h1T")
                    nc.scalar.copy(h1T[:], pT[:])
                    nc.tensor.matmul(ps2[:], lhsT=h1T[:], rhs=w2_t[:, cch, :],
                                     start=(cch == 0), stop=(cch == 1))
                hg = ring.tile([128, OUT_CH], dt.float32, tag="hg")
                nc.vector.tensor_tensor(out=hg[:], in0=ps2[:],
                                        in1=b2_t[:],
                                        op=OP.add)
                nc.vector.tensor_scalar_mul(u_t[:, g, :], hg[:], dinv_t[:, g:g + 1])

            nc.vector.tensor_scalar_mul(ahd_t[:].rearrange("p g f -> p (g f)"),
                                        u_t[:].rearrange("p g f -> p (g f)"), ALPHA)
            nc.vector.memset(s2f_t[:].rearrange("p g f -> p (g f)"), 1.0 - ALPHA)
            for g in range(NB):
                nc.vector.tensor_scalar_mul(s2f_t[:, g, :], s2f_t[:, g, :],
                                            dinv2_t[:, g:g + 1])

            def share(step):
                tab = tabs[step % 2]
                nc.sync.dma_start(
                    bounce.ap().rearrange("(g p) f -> p g f", p=128)[:, :, :OUT_CH],
                    u_t[:])
                nc.gpsimd.collective_compute(
                    "AllGather", mybir.AluOpType.bypass,
                    replica_groups=[list(range(C))],
                    ins=[bounce.ap()],
                    outs=[tab.ap().rearrange("s (q f) -> (s q) f", f=F64)[:R_TOT]],
                )
            share(0)

            for step in range(1, K_STEPS + 1):
                tab = tabs[(step - 1) % 2]
                nc.vector.memset(r_t[:].rearrange("p g f -> p (g f)"), 0.0)
                for (slots, ch, colbase) in chunk_meta:
                    gbuf = ring.tile([NLANE, CHUNK_SLOTS, 256], dt.float32, tag="gbuf")
                    ixt = ring.tile([NLANE, CHUNK_SLOTS * 8], dt.int16, tag="ixt")
                    nc.sync.dma_start(ixt[:, :slots * 8],
                                      idx_in.ap()[:, colbase:colbase + slots * 8])
                    done = 0
                    while done < slots:
                        k = min(8, slots - done)
                        ni = k * 128
                        nc.gpsimd.dma_gather(
                            gbuf[:, done:done + k, :], tab.ap(),
                            ixt[:, done * 8:done * 8 + ni // 16],
                            ni, ni, 256, single_packet=True)
                        done += k
                    m0 = 0
                    for pi in ch:
                        j, d = pieces[pi]
                        pc = piece_col[pi]
                        r4 = ring.tile([NLANE, 4, OUT_CH], dt.float32, tag="r4")
                        tmpk = ring.tile([NLANE, MAX_D, OUT_CH], dt.float32, tag="tmpk")
                        for kcls in range(4):
                            nc.vector.tensor_tensor(
                                out=tmpk[:, :d, :],
                                in0=gbuf[:, m0:m0 + d, kcls * 64:kcls * 64 + OUT_CH],
                                in1=msk_t[:, pc:pc + d, kcls:kcls + 1]
                                    .to_broadcast([NLANE, d, OUT_CH]),
                                op=OP.mult)
                            nc.vector.tensor_reduce(
                                out=r4[:, kcls, :],
                                in_=tmpk[:, :d, :].rearrange("p d f -> p f d"),
                                axis=mybir.AxisListType.X, op=OP.add)
                        rsum = ring.tile([NLANE, OUT_CH], dt.float32, tag="rsum")
                        nc.vector.tensor_reduce(
                            out=rsum[:], in_=r4[:].rearrange("p c f -> p f c"),
                            axis=mybir.AxisListType.X, op=OP.add)
                        nc.vector.tensor_tensor(out=r_t[:, j, :], in0=r_t[:, j, :],
                                                in1=rsum[:], op=OP.add)
                        m0 += d
                uf = u_t[:].rearrange("p g f -> p (g f)")
                rf = r_t[:].rearrange("p g f -> p (g f)")
                nc.vector.tensor_tensor(out=rf, in0=rf, in1=uf, op=OP.add)
                nc.vector.tensor_tensor(out=rf, in0=rf,
                                        in1=s2f_t[:].rearrange("p g f -> p (g f)"),
                                        op=OP.mult)
                nc.vector.tensor_tensor(out=uf, in0=rf,
                                        in1=ahd_t[:].rearrange("p g f -> p (g f)"),
                                        op=OP.add)
                if step < K_STEPS:
                    share(step)

            zout = pool.tile([NLANE, NB, OUT_CH], dt.float32)
            for g in range(NB):
                nc.vector.tensor_scalar_mul(zout[:, g, :], u_t[:, g, :],
                                            sdeg_t[:, g:g + 1])
            nc.sync.dma_start(out_d.ap().rearrange("(g p) f -> p g f", p=128),
                              zout[:])

    nc.compile()
    return nc


def kernel(x, edge_index, W1, b1, W2, b2):
    import concourse.bass_utils as bass_utils

    x = np.asarray(x, dtype=np.float32)
    W1 = np.asarray(W1, dtype=np.float32)
    b1 = np.tile(np.asarray(b1, dtype=np.float32).reshape(1, -1), (128, 1))
    W2 = np.asarray(W2, dtype=np.float32)
    b2 = np.tile(np.asarray(b2, dtype=np.float32).reshape(1, -1), (128, 1))

    pre = _preprocess(edge_index)
    nc = _build_program(pre)

    order = pre["order"]
    ident = np.eye(128, dtype=np.float32)
    in_maps = []
    for c in range(C):
        s = np.arange(S_PER_CORE)
        r = 8 * s + c
        valid = r < N_NODES
        old = np.where(valid, order[np.minimum(r, N_NODES - 1)], 0)
        xs = x[old]
        xs[~valid] = 0.0
        xt = np.ascontiguousarray(xs.reshape(NB, 128, 4, 128).transpose(0, 2, 3, 1))
        in_maps.append({
            "xt": xt, "w1": W1, "b1": b1, "w2": W2, "b2": b2,
            "deg": pre["deg"][c],
            "msk": np.ascontiguousarray(pre["masks"][c].reshape(NLANE, -1)),
            "idx": np.ascontiguousarray(pre["idx_wrapped"][c]),
            "ident": ident,
        })

    res = bass_utils.run_bass_kernel_spmd(nc, in_maps, core_ids=list(range(C)))

    out = np.zeros((N_NODES, OUT_CH), dtype=np.float32)
    for c in range(C):
        z = res.results[c]["out"]
        s = np.arange(S_PER_CORE)
        r = 8 * s + c
        valid = r < N_NODES
        out[order[r[valid]]] = z[valid]
    return out
